# revision 1
# baseline (speedup 1.0000x reference)
"""CondInst dynamic mask head on 8 Trainium2 NeuronCores.

Math (per instance i with gathered params):
    x_i   = [rel_i (2,HW); feats_b (8,HW)]
    h1    = relu(w0_i @ x_i + b0_i)        # (8,HW)
    h2    = relu(w1_i @ h1 + b1_i)         # (8,HW)
    out_i = sigmoid(w2_i @ h2 + b2_i)      # (1,HW)

rel_i = (loc_i - coords)/128 is affine in the shared coords map, so it is
folded into a shared X = [coords/128; feats; ones] with per-instance
effective weights Ahat_i = [-w0r_i | w0f_i] and bias c0_i = b0_i + w0r_i@loc_i/128
(the bias rides the ones-row of X).

Sharding: core c -> batch b=c//2, L-half c%2 (8192 cols), all 100 instances.
Instances are grouped in 25 blocks of 4 (32 rows of 4x8 channels) mapped onto
32x32 PE-array tiles; matmuls run in float32r.
"""

import os
import sys

import numpy as np

sys.path.insert(0, "/opt/trn_rl_repo")
os.environ.setdefault("MYCRO_LOCAL_CACHE", "1")

B, K, C, H, Wd = 4, 100, 8, 128, 128
HW = H * Wd
P_ = (C + 2) * C + C + C * C + C + C + 1  # 169
LC = HW // 2          # 8192 L-columns per core
WCH = 256             # L-chunk (free dim) per matmul / psum round
NCHUNK = LC // WCH    # 32
NB = 25               # instance blocks of 4
NCORE = 8

MM_DTYPE = os.environ.get("CONDINST_MM_DTYPE", "bfloat16")

_PROGRAM = None  # cached (nc, meta)


# ---------------------------------------------------------------- mappings
def l0_map(t):
    """block t -> (row_group r, col_group c, slot s, stationary col-block k)."""
    if t < 16:
        r, c, s = t // 4, t % 4, 0
    else:
        u = t - 16
        r, c, s = u // 4, u % 4, 1
    return r, c, s, c + 4 * s


def l1_map(t):
    """block t -> (psum bank-group q=row_group, col_group c1, slot s, col-block k1)."""
    if t < 16:
        q, c1, s = t % 4, t // 4, 0
    else:
        u = t - 16
        q, c1, s = u % 4, u // 4, 1
    return q, c1, s, c1 + 4 * s


def l2_map(t):
    """block t -> (chain v = c1 of L1 = row AND col group, chain position w).

    L2 accumulates chain v's blocks into psum bank v, partitions
    [32v + 4w + l]; all of chain v's matmuls run on sub-array (v, v) so the
    accumulation chain is naturally serialized on one PE tile position.
    """
    _, c1, _, _ = l1_map(t)
    w = t % 4 if t < 16 else 4 + (t - 16) % 4
    return c1, w


CHAINS = [[] for _ in range(4)]
for _t in range(NB):
    CHAINS[l2_map(_t)[0]].append(_t)
# chain v rows [32v + 4w + l] -> inst 4*CHAINS[v][w] + l
# out-DMA runs: (sbuf row start, out row start, nrows)
OUT_RUNS = []
for _v in range(4):
    _n = len(CHAINS[_v])
    OUT_RUNS.append((32 * _v, 16 * _v, 16))
    if _n > 4:
        OUT_RUNS.append((32 * _v + 16, 64 + 16 * _v, 4 * (_n - 4)))


# ---------------------------------------------------------------- host prep
def _prep_inputs(seg_feat, conv_weight, ind):
    seg_feat = np.asarray(seg_feat, dtype=np.float32)
    conv_weight = np.asarray(conv_weight, dtype=np.float32)
    ind_np = np.asarray(ind)
    ind64 = ind_np.astype(np.int64)

    cw = conv_weight.reshape(B, P_, HW)
    # params[b, k, p] = cw[b, p, ind[b, k]]
    params = np.take_along_axis(cw, ind64[:, None, :], axis=2)  # [B, P, K]
    params = params.transpose(0, 2, 1)  # [B, K, P]

    w0 = params[..., 0:80].reshape(B, K, C, C + 2)
    w1 = params[..., 80:144].reshape(B, K, C, C)
    w2 = params[..., 144:152].reshape(B, K, 1, C)
    b0 = params[..., 152:160]
    b1 = params[..., 160:168]
    b2 = params[..., 168:169]

    xi = (ind64 % Wd).astype(np.float32)
    yi = (ind64 // Wd).astype(np.float32)
    loc = np.stack([xi, yi], axis=-1)  # [B, K, 2]

    w0r = w0[..., 0:2]   # [B, K, 8, 2]
    w0f = w0[..., 2:10]  # [B, K, 8, 8]
    ahat = np.concatenate([-w0r, w0f], axis=-1)  # [B, K, 8, 10]
    c0 = b0 + np.einsum("bkoc,bkc->bko", w0r, loc) / 128.0  # [B, K, 8]

    lin = np.arange(HW, dtype=np.float32)
    coords_x = (lin % Wd) / 128.0
    coords_y = np.floor(lin / Wd) / 128.0

    in_maps = []
    for core in range(NCORE):
        b = core // 2
        lo = (core % 2) * LC
        sl = slice(lo, lo + LC)

        xrep = np.empty((11, LC), dtype=np.float32)
        xrep[0] = coords_x[sl]
        xrep[1] = coords_y[sl]
        xrep[2:10] = seg_feat[b].reshape(C, HW)[:, sl]
        xrep[10] = 1.0

        w0s = np.zeros((128, 256), dtype=np.float32)
        w1s = np.zeros((128, 256), dtype=np.float32)
        w2s = np.zeros((128, 256), dtype=np.float32)
        b1sb = np.zeros((128, 8), dtype=np.float32)
        b2sb = np.zeros((128, 1), dtype=np.float32)

        for t in range(NB):
            r0, _, _, k0 = l0_map(t)
            q1, c1, s1, k1 = l1_map(t)
            v2, w2i = l2_map(t)
            for j in range(4):
                inst = 4 * t + j
                # L0 stationary: [11, 32] at rows 32r0, cols 32k0 (+8j per inst)
                w0s[32 * r0:32 * r0 + 10, 32 * k0 + 8 * j:32 * k0 + 8 * j + 8] = \
                    ahat[b, inst].T
                w0s[32 * r0 + 10, 32 * k0 + 8 * j:32 * k0 + 8 * j + 8] = c0[b, inst]
                # L1 stationary: blockdiag W1^T at rows 32q1
                w1s[32 * q1 + 8 * j:32 * q1 + 8 * j + 8,
                    32 * k1 + 8 * j:32 * k1 + 8 * j + 8] = w1[b, inst].T
                # L1 bias vector for psum partition 32c1 + 8j + ch, column 2q1+s1
                b1sb[32 * c1 + 8 * j:32 * c1 + 8 * j + 8, 2 * q1 + s1] = b1[b, inst]
                # L2 stationary slab [32v2, 32*w2i]: nonzero col 4*w2i+j
                w2s[32 * v2 + 8 * j:32 * v2 + 8 * j + 8,
                    32 * w2i + 4 * w2i + j] = w2[b, inst, 0]
                # sigmoid bias: psum partition 32v2 + 4*w2i + j = inst
                b2sb[32 * v2 + 4 * w2i + j, 0] = b2[b, inst, 0]

        if MM_DTYPE == "bfloat16":
            import ml_dtypes
            bf16 = ml_dtypes.bfloat16
            xrep = xrep.astype(bf16)
            w0s = w0s.astype(bf16)
            w1s = w1s.astype(bf16)
            w2s = w2s.astype(bf16)
        in_maps.append({
            "xrep": xrep, "w0s": w0s, "w1s": w1s, "w2s": w2s,
            "b1sb": b1sb, "b2sb": b2sb,
        })
    return in_maps, ind_np.dtype


# ---------------------------------------------------------------- program
def build_program():
    global _PROGRAM
    if _PROGRAM is not None:
        return _PROGRAM

    import concourse.tile as tile
    from concourse import bacc, mybir

    nc = bacc.Bacc("TRN2", target_bir_lowering=False, debug=False)
    f32 = mybir.dt.float32
    mm_dt = getattr(mybir.dt, MM_DTYPE)
    Relu = mybir.ActivationFunctionType.Relu
    Sigmoid = mybir.ActivationFunctionType.Sigmoid
    Alu = mybir.AluOpType

    xrep_h = nc.dram_tensor("xrep", [11, LC], mm_dt, kind="ExternalInput")
    w0s_h = nc.dram_tensor("w0s", [128, 256], mm_dt, kind="ExternalInput")
    w1s_h = nc.dram_tensor("w1s", [128, 256], mm_dt, kind="ExternalInput")
    w2s_h = nc.dram_tensor("w2s", [128, 256], mm_dt, kind="ExternalInput")
    b1_h = nc.dram_tensor("b1sb", [128, 8], f32, kind="ExternalInput")
    b2_h = nc.dram_tensor("b2sb", [128, 1], f32, kind="ExternalInput")
    out_h = nc.dram_tensor("out_shard", [4 * NB, LC], f32, kind="ExternalOutput")


    with tile.TileContext(nc) as tc:
        with (
            tc.tile_pool(name="const", bufs=1) as cpool,
            tc.tile_pool(name="h1p", bufs=8) as h1pool,
            tc.tile_pool(name="h2p", bufs=8) as h2pool,
            tc.tile_pool(name="osp", bufs=3) as ospool,
            tc.tile_pool(name="ps", bufs=4, space="PSUM") as pspool,
        ):
            xrep = cpool.tile([128, LC], mm_dt, tag="xrep")
            w0s = cpool.tile([128, 256], mm_dt, tag="w0s")
            w1s = cpool.tile([128, 256], mm_dt, tag="w1s")
            w2s = cpool.tile([128, 256], mm_dt, tag="w2s")
            b1s = cpool.tile([128, 8], f32, tag="b1s")
            b2s = cpool.tile([128, 1], f32, tag="b2s")

            for r in range(4):
                nc.gpsimd.dma_start(xrep[32 * r:32 * r + 11, :], xrep_h[:])
            nc.gpsimd.dma_start(w0s[:], w0s_h[:])
            nc.gpsimd.dma_start(w1s[:], w1s_h[:])
            nc.gpsimd.dma_start(w2s[:], w2s_h[:])
            nc.gpsimd.dma_start(b1s[:], b1_h[:])
            nc.gpsimd.dma_start(b2s[:], b2_h[:])

            # Software-pipelined emission: in iteration `it` the PE stream is
            # [L0 MMs of chunk it][L1 MMs of chunk it-1][L2 MMs of chunk it-2]
            # so every matmul's inputs were evacuated a full stage earlier and
            # PE / ScalarE / VectorE all run without cross-stage stalls.
            h1_by, h2_by, p2_by = {}, {}, {}
            rounds = [(wi, v) for wi in range(max(len(c) for c in CHAINS))
                      for v in range(4) if wi < len(CHAINS[v])]

            for it in range(NCHUNK + 3):
                a, b2c, c2c = it, it - 1, it - 2

                # ---- L0 matmuls for chunk a
                if a < NCHUNK:
                    fl = slice(a * WCH, (a + 1) * WCH)
                    p0 = [pspool.tile([128, 512], f32, tag="pp", bufs=7,
                                      name=f"p0_{a}_{r}") for r in range(4)]
                    for t in range(NB):
                        r0, c0_, s0, k0 = l0_map(t)
                        nc.tensor.matmul(
                            p0[r0][32 * c0_:32 * c0_ + 32,
                                   256 * s0:256 * s0 + 256],
                            (w0s[32 * r0:32 * r0 + 11, 32 * k0:32 * k0 + 32]),
                            (xrep[32 * r0:32 * r0 + 11, fl]),
                            tile_position=(32 * r0, 32 * c0_),
                        )

                # ---- sigmoid + store for the pair ending at chunk it-3
                d = it - 3
                if 0 <= d < NCHUNK and d % 2 == 1:
                    p2 = p2_by.pop(d - 1)
                    fl2 = slice((d - 1) * WCH, (d + 1) * WCH)
                    os_t = ospool.tile([128, 512], f32, tag="os",
                                       name=f"os_{d}")
                    for v in range(4):
                        nr = 4 * len(CHAINS[v])
                        nc.scalar.activation(
                            os_t[32 * v:32 * v + nr, :],
                            p2[32 * v:32 * v + nr, :], Sigmoid,
                            bias=b2s[32 * v:32 * v + nr, :])
                    for (srow, orow, nrow) in OUT_RUNS:
                        nc.gpsimd.dma_start(out_h[orow:orow + nrow, fl2],
                                            os_t[srow:srow + nrow, :])

                # ---- L1 matmuls for chunk b2c
                if 0 <= b2c < NCHUNK:
                    h1 = h1_by[b2c]
                    p1 = [pspool.tile([128, 512], f32, tag="pp", bufs=7,
                                      name=f"p1_{b2c}_{q}") for q in range(4)]
                    for t in range(NB):
                        r0, c0_, s0, _ = l0_map(t)
                        q1, c1, s1, k1 = l1_map(t)
                        nc.tensor.matmul(
                            p1[q1][32 * c1:32 * c1 + 32,
                                   256 * s1:256 * s1 + 256],
                            (w1s[32 * q1:32 * q1 + 32, 32 * k1:32 * k1 + 32]),
                            (h1[r0][32 * c0_:32 * c0_ + 32,
                                    256 * s0:256 * s0 + 256]),
                            tile_position=(32 * q1, 32 * c1),
                        )

                # ---- h1 = relu(p0) evacuation for chunk a
                if a < NCHUNK:
                    h1 = [h1pool.tile([128, 512], mm_dt, tag=f"h1_{r}", bufs=3,
                                      name=f"h1_{a}_{r}") for r in range(4)]
                    h1_by[a] = h1
                    nc.scalar.activation(h1[0][:], p0[0][:], Relu)
                    nc.vector.tensor_scalar_max(h1[1][:], p0[1][:], 0.0)
                    nc.scalar.activation(h1[2][:, 0:256], p0[2][:, 0:256], Relu)
                    nc.vector.tensor_scalar_max(h1[2][0:32, 256:512],
                                                p0[2][0:32, 256:512], 0.0)
                    nc.vector.tensor_scalar_max(h1[3][:, 0:256],
                                                p0[3][:, 0:256], 0.0)

                # ---- L2 matmuls for chunk c2c (4 chains, round-robin)
                if 0 <= c2c < NCHUNK:
                    pair = c2c % 2
                    if pair == 0:
                        p2_by[c2c] = pspool.tile([128, 512], f32, tag="pc",
                                                 bufs=1, name=f"p2_{c2c}")
                    p2 = p2_by[c2c - pair]
                    h2 = h2_by.pop(c2c)
                    for (wi, v) in rounds:
                        t = CHAINS[v][wi]
                        q1, c1, s1, _ = l1_map(t)
                        nc.tensor.matmul(
                            p2[32 * v:32 * v + 32, 256 * pair:256 * pair + 256],
                            (w2s[32 * v:32 * v + 32, 32 * wi:32 * wi + 32]),
                            (h2[q1][32 * c1:32 * c1 + 32,
                                    256 * s1:256 * s1 + 256]),
                            start=(wi == 0), stop=(wi == len(CHAINS[v]) - 1),
                            skip_group_check=True,
                            tile_position=(32 * v, 32 * v),
                        )

                # ---- h2 = relu(p1 + b1) evacuation for chunk b2c
                if 0 <= b2c < NCHUNK:
                    h2 = [h2pool.tile([128, 512], mm_dt, tag=f"h2_{q}", bufs=3,
                                      name=f"h2_{b2c}_{q}") for q in range(4)]
                    h2_by[b2c] = h2
                    s1parts = [96, 64, 64, 64]
                    for q in range(4):
                        bias0 = b1s[:, 2 * q:2 * q + 1]
                        if q < 2:
                            nc.scalar.activation(h2[q][:, 0:256],
                                                 p1[q][:, 0:256], Relu,
                                                 bias=bias0)
                        else:
                            nc.vector.tensor_scalar(
                                h2[q][:, 0:256], p1[q][:, 0:256],
                                bias0, 0.0, Alu.add, Alu.max)
                        np1 = s1parts[q]
                        bias1 = b1s[0:np1, 2 * q + 1:2 * q + 2]
                        if q == 0:
                            nc.scalar.activation(h2[q][0:np1, 256:512],
                                                 p1[q][0:np1, 256:512], Relu,
                                                 bias=bias1)
                        else:
                            nc.vector.tensor_scalar(
                                h2[q][0:np1, 256:512], p1[q][0:np1, 256:512],
                                bias1, 0.0, Alu.add, Alu.max)

    nc.compile()
    _PROGRAM = nc
    return nc


# ---------------------------------------------------------------- entry
def kernel(seg_feat, conv_weight, ind):
    from concourse.bass_utils import run_bass_kernel_spmd

    in_maps, ind_dtype = _prep_inputs(seg_feat, conv_weight, ind)
    nc = build_program()
    res = run_bass_kernel_spmd(nc, in_maps, list(range(NCORE)))
    out = np.empty((B, K, HW), dtype=np.float32)
    for core in range(NCORE):
        b = core // 2
        lo = (core % 2) * LC
        out[b, :, lo:lo + LC] = res.results[core]["out_shard"]
    return out.reshape(B, K, H, Wd)



# revision 3
# speedup vs baseline: 1.1853x; 1.1853x over previous
"""CondInst dynamic mask head on 8 Trainium2 NeuronCores.

Math (per instance i with gathered params):
    x_i   = [rel_i (2,HW); feats_b (8,HW)]
    h1    = relu(w0_i @ x_i + b0_i)        # (8,HW)
    h2    = relu(w1_i @ h1 + b1_i)         # (8,HW)
    out_i = sigmoid(w2_i @ h2 + b2_i)      # (1,HW)

rel_i is affine in the shared coords map, so it folds into a shared
X = [coords/128; feats; ones] with per-instance effective weights
Ahat_i = [-w0r_i | w0f_i] and bias c0_i riding the ones-row.

Sharding: core c -> batch b=c//2, L-half c%2 (8192 cols), all 100 instances.

Layout: 7 slabs of 16 instances (slab 6 holds 4).  Each slab's activations
live in one [128, 1024]-bf16 tile (16 inst x 8 ch rows).  All matmuls write
bf16 PSUM (full 2KB bank at N=1024) so the PSUM->SBUF evacuations run in the
DVE 2x_1P perf mode; evac work is split DVE/ACT to balance both engines,
which are the roofline for this kernel.
"""

import os
import sys

import numpy as np

sys.path.insert(0, "/opt/trn_rl_repo")
os.environ.setdefault("MYCRO_LOCAL_CACHE", "1")

B, K, C, H, Wd = 4, 100, 8, 128, 128
HW = H * Wd
LC = HW // 2          # 8192 cols per core
WCH = 512             # chunk (free dim) per matmul / psum bank (512 f32)
NCHUNK = LC // WCH    # 16
NSLAB = 7             # slabs of 16 instances (last has 4)
NCORE = 8

# evac engine pattern per chunk: 14 evacs (7 E1 + 7 E2 interleaved),
# 'D' = VectorE (tensor_scalar, 2x bf16), 'A' = ScalarE (activation)
EVAC_PATTERN = os.environ.get(
    "CONDINST_EVAC", "DADADADDADADAD")

_PROGRAM = None  # cached nc


def _nblocks(s):
    return 4 if s < 6 else 1


# ---------------------------------------------------------------- host prep
def _prep_inputs(seg_feat, conv_weight, ind):
    import ml_dtypes
    bf16 = ml_dtypes.bfloat16

    seg_feat = np.asarray(seg_feat, dtype=np.float32)
    conv_weight = np.asarray(conv_weight, dtype=np.float32)
    ind64 = np.asarray(ind).astype(np.int64)

    cw = conv_weight.reshape(B, -1, HW)
    params = np.take_along_axis(cw, ind64[:, None, :], axis=2)  # [B, P, K]
    params = params.transpose(0, 2, 1)  # [B, K, P]

    w0 = params[..., 0:80].reshape(B, K, C, C + 2)
    w1 = params[..., 80:144].reshape(B, K, C, C)
    w2 = params[..., 144:152].reshape(B, K, C)
    b0 = params[..., 152:160]
    b1 = params[..., 160:168]
    b2 = params[..., 168]

    xi = (ind64 % Wd).astype(np.float32)
    yi = (ind64 // Wd).astype(np.float32)
    loc = np.stack([xi, yi], axis=-1)  # [B, K, 2]

    w0r = w0[..., 0:2]   # [B, K, 8, 2]
    w0f = w0[..., 2:10]  # [B, K, 8, 8]
    ahat = np.concatenate([-w0r, w0f], axis=-1)  # [B, K, 8, 10]
    c0 = b0 + np.einsum("bkoc,bkc->bko", w0r, loc) / 128.0  # [B, K, 8]

    lin = np.arange(HW, dtype=np.float32)
    coords_x = (lin % Wd) / 128.0
    coords_y = np.floor(lin / Wd) / 128.0

    in_maps = []
    for core in range(NCORE):
        b = core // 2
        sl = slice((core % 2) * LC, (core % 2) * LC + LC)

        xrep = np.zeros((128, LC), dtype=np.float32)
        for r in range(4):
            xrep[32 * r + 0] = coords_x[sl]
            xrep[32 * r + 1] = coords_y[sl]
            xrep[32 * r + 2:32 * r + 10] = seg_feat[b].reshape(C, HW)[:, sl]
            xrep[32 * r + 10] = 1.0

        w0sb = np.zeros((128, 128 * NSLAB), dtype=np.float32)
        w1sb = np.zeros((128, 32 * NSLAB), dtype=np.float32)
        w2sb = np.zeros((128, 32 * NSLAB), dtype=np.float32)
        b1sb = np.zeros((128, NSLAB), dtype=np.float32)
        b2sb = np.zeros((128, 1), dtype=np.float32)

        for s in range(NSLAB):
            side = s % 2
            for cb in range(4):          # out/row block within slab
                for jj in range(4):
                    inst = 16 * s + 4 * cb + jj
                    if inst >= K:
                        continue
                    # L0: out block cb at row-group r0, stationary [11, 32]
                    r0 = (cb + s) % 4
                    col = 128 * s + 32 * cb + 8 * jj
                    w0sb[32 * r0:32 * r0 + 10, col:col + 8] = ahat[b, inst].T
                    w0sb[32 * r0 + 10, col:col + 8] = c0[b, inst]
                    # L1: contract block r=cb -> out col group c1
                    c1 = (cb + s) % 4
                    w1sb[32 * cb + 8 * jj:32 * cb + 8 * jj + 8,
                         32 * s + 8 * jj:32 * s + 8 * jj + 8] = w1[b, inst].T
                    # h2 partition p = 32*c1 + 8*jj + co holds (inst, co)
                    b1sb[32 * c1 + 8 * jj:32 * c1 + 8 * jj + 8, s] = b1[b, inst]
                    # L2: out partition 32*(s//2) + 16*side + q,  q = 4*cb+jj
                    q = 4 * cb + jj
                    w2sb[32 * c1 + 8 * jj:32 * c1 + 8 * jj + 8,
                         32 * s + 16 * side + q] = w2[b, inst]
        b2sb[:K, 0] = b2[b]

        in_maps.append({
            "xrep": xrep.astype(bf16),
            "w0sb": w0sb.astype(bf16),
            "w1sb": w1sb.astype(bf16),
            "w2sb": w2sb.astype(bf16),
            "b1sb": b1sb, "b2sb": b2sb,
        })
    return in_maps, None


# ---------------------------------------------------------------- program
def build_program():
    global _PROGRAM
    if _PROGRAM is not None:
        return _PROGRAM

    import concourse.tile as tile
    from concourse import bacc, mybir

    nc = bacc.Bacc("TRN2", target_bir_lowering=False, debug=False)
    f32 = mybir.dt.float32
    bf16 = mybir.dt.bfloat16
    Relu = mybir.ActivationFunctionType.Relu
    Sigmoid = mybir.ActivationFunctionType.Sigmoid
    Alu = mybir.AluOpType

    xrep_h = nc.dram_tensor("xrep", [128, LC], bf16, kind="ExternalInput")
    w0_h = nc.dram_tensor("w0sb", [128, 128 * NSLAB], bf16, kind="ExternalInput")
    w1_h = nc.dram_tensor("w1sb", [128, 32 * NSLAB], bf16, kind="ExternalInput")
    w2_h = nc.dram_tensor("w2sb", [128, 32 * NSLAB], bf16, kind="ExternalInput")
    b1_h = nc.dram_tensor("b1sb", [128, NSLAB], f32, kind="ExternalInput")
    b2_h = nc.dram_tensor("b2sb", [128, 1], f32, kind="ExternalInput")
    out_h = nc.dram_tensor("out_shard", [K, LC], f32, kind="ExternalOutput")

    with tile.TileContext(nc) as tc:
        with (
            tc.tile_pool(name="const", bufs=1) as cpool,
            tc.tile_pool(name="h1p", bufs=3) as h1pool,
            tc.tile_pool(name="h2p", bufs=3) as h2pool,
            tc.tile_pool(name="osp", bufs=2) as ospool,
            tc.tile_pool(name="ps0", bufs=3, space="PSUM") as p0pool,
            tc.tile_pool(name="ps1", bufs=3, space="PSUM") as p1pool,
            tc.tile_pool(name="ps2", bufs=2, space="PSUM") as p2pool,
        ):
            xr = cpool.tile([128, LC], bf16, tag="xr")
            w0 = cpool.tile([128, 128 * NSLAB], bf16, tag="w0")
            w1 = cpool.tile([128, 32 * NSLAB], bf16, tag="w1")
            w2 = cpool.tile([128, 32 * NSLAB], bf16, tag="w2")
            b1 = cpool.tile([128, NSLAB], f32, tag="b1")
            b2 = cpool.tile([128, 1], f32, tag="b2")

            nc.gpsimd.dma_start(w0[:], w0_h[:])
            nc.gpsimd.dma_start(w1[:], w1_h[:])
            nc.gpsimd.dma_start(w2[:], w2_h[:])
            nc.gpsimd.dma_start(b1[:], b1_h[:])
            nc.gpsimd.dma_start(b2[:], b2_h[:])
            for k in range(NCHUNK):
                nc.gpsimd.dma_start(xr[:, WCH * k:WCH * (k + 1)],
                                    xrep_h[:, WCH * k:WCH * (k + 1)])

            NJOB = NCHUNK * NSLAB  # 56
            p0t, h1t, p1t, h2t = {}, {}, {}, {}
            p2t, ost = {}, {}
            eci = 0  # evac counter for engine pattern

            def evac(dst, src, bias, tag):
                nonlocal eci
                eng = EVAC_PATTERN[eci % len(EVAC_PATTERN)]
                eci += 1
                if eng == "D":
                    if bias is None:
                        nc.vector.tensor_scalar_max(dst, src, 0.0)
                    else:
                        nc.vector.tensor_scalar(dst, src, bias, 0.0,
                                                Alu.add, Alu.max)
                else:
                    if bias is None:
                        nc.scalar.activation(dst, src, Relu)
                    else:
                        nc.scalar.activation(dst, src, Relu, bias=bias)

            for it in range(NJOB + 4):
                # ---- stage A: L0 matmuls for job `it`
                j = it
                if j < NJOB:
                    k, s = divmod(j, NSLAB)
                    fl = slice(WCH * k, WCH * (k + 1))
                    p0 = p0pool.tile([128, WCH], f32, tag="p0",
                                     name=f"p0_{j}")
                    p0t[j] = (p0, k, s)
                    for cb in range(_nblocks(s)):
                        r0 = (cb + s) % 4
                        nc.tensor.matmul(
                            p0[32 * cb:32 * cb + 32, :],
                            w0[32 * r0:32 * r0 + 11,
                               128 * s + 32 * cb:128 * s + 32 * cb + 32],
                            xr[32 * r0:32 * r0 + 11, fl],
                            tile_position=(32 * r0, 32 * cb),
                        )

                # ---- stage E1: evac p0 -> h1 for job it-1
                j = it - 1
                if 0 <= j < NJOB:
                    p0, k, s = p0t.pop(j)
                    nr = 32 * _nblocks(s)
                    h1 = h1pool.tile([128, WCH], bf16, tag="h1",
                                     name=f"h1_{j}")
                    h1t[j] = h1
                    evac(h1[0:nr, :], p0[0:nr, :], None, "e1")

                # ---- stage B: L1 matmuls for job it-2
                j = it - 2
                if 0 <= j < NJOB:
                    k, s = divmod(j, NSLAB)
                    h1 = h1t.pop(j)
                    p1 = p1pool.tile([128, WCH], f32, tag="p1",
                                     name=f"p1_{j}")
                    p1t[j] = p1
                    for r in range(_nblocks(s)):
                        c1 = (r + s) % 4
                        nc.tensor.matmul(
                            p1[32 * c1:32 * c1 + 32, :],
                            w1[32 * r:32 * r + 32, 32 * s:32 * s + 32],
                            h1[32 * r:32 * r + 32, :],
                            tile_position=(32 * r, 32 * c1),
                        )

                # ---- stage E2: evac p1 + b1 -> h2 for job it-3
                j = it - 3
                if 0 <= j < NJOB:
                    k, s = divmod(j, NSLAB)
                    p1 = p1t.pop(j)
                    h2 = h2pool.tile([128, WCH], bf16, tag="h2",
                                     name=f"h2_{j}")
                    h2t[j] = h2
                    if s < 6:
                        evac(h2[:], p1[:], b1[:, s:s + 1], "e2")
                    else:
                        c1 = (0 + s) % 4  # only row-block 0 is live
                        evac(h2[32 * c1:32 * c1 + 32, :],
                             p1[32 * c1:32 * c1 + 32, :],
                             b1[32 * c1:32 * c1 + 32, s:s + 1], "e2")

                # ---- stage C: L2 matmul for job it-4 (+ sigmoid at chunk end)
                j = it - 4
                if 0 <= j < NJOB:
                    k, s = divmod(j, NSLAB)
                    h2 = h2t.pop(j)
                    if s == 0:
                        p2t[k] = p2pool.tile([128, WCH], f32, tag="p2",
                                             name=f"p2_{k}")
                    g = s // 2
                    nc.tensor.matmul(
                        p2t[k][32 * g:32 * g + 32, :],
                        w2[:, 32 * s:32 * s + 32],
                        h2[:],
                        start=(s % 2 == 0), stop=(s % 2 == 1 or s == 6),
                        skip_group_check=True,
                        tile_position=(0, 32 * g),
                    )
                    if s == NSLAB - 1:
                        fl = slice(WCH * k, WCH * (k + 1))
                        os_t = ospool.tile([128, WCH], f32, tag="os",
                                           name=f"os_{k}")
                        p2 = p2t.pop(k)
                        nc.scalar.activation(os_t[0:K, :], p2[0:K, :],
                                             Sigmoid, bias=b2[0:K, :])
                        nc.gpsimd.dma_start(out_h[:, fl], os_t[0:K, :])

    nc.compile()
    _PROGRAM = nc
    return nc


# ---------------------------------------------------------------- entry
def kernel(seg_feat, conv_weight, ind):
    from concourse.bass_utils import run_bass_kernel_spmd

    in_maps, _ = _prep_inputs(seg_feat, conv_weight, ind)
    nc = build_program()
    res = run_bass_kernel_spmd(nc, in_maps, list(range(NCORE)))
    out = np.empty((B, K, HW), dtype=np.float32)
    for core in range(NCORE):
        b = core // 2
        lo = (core % 2) * LC
        out[b, :, lo:lo + LC] = res.results[core]["out_shard"]
    return out.reshape(B, K, H, Wd)
